# revision 1
# baseline (speedup 1.0000x reference)
"""LookupConv2d Trainium2 kernel.

Math: out = conv2d(x, W), W[o] = sum_s coeff[o,s] * dictionary[idx[o,s]].
Factorization: W = M @ D where M[o,d] = sum_{s: idx[o,s]=d} coeff[o,s] is a
(512, 100) scatter of the coefficients.  Then
    out = M @ conv2d(x, dictionary)
i.e. a 100-channel conv (23 GFLOP) followed by a 1x1 512x100 mix (5 GFLOP)
instead of a 512-channel conv (118 GFLOP) -- 4.2x fewer FLOPs.

Precision: the TensorE f32r mode streams 1 row/cycle (4x faster than fp32
mode) but rounds operands to 12 significant bits (RNE, measured on HW).
We split x and the dictionary into exact 12-bit halves (xh = top 12 bits,
xl = remainder, both f32r-invariant) and accumulate
    xh*wh + xl*wh + xh*wl
in fp32 PSUM -- full fp32-class accuracy (only xl*wl ~ 2^-24 dropped) at
3 cycles/row instead of fp32 mode's 4.  The small 1x1 mix stays in native
fp32 mode.

Sharding: data-parallel over batch N=16 -> 2 images per core on 8 cores.
dictionary (as [128,100] lhsT tap matrices) and M^T are replicated.
"""

import numpy as np

N_CORES = 8
IMGS_PER_CORE = 2
CIN = 256
COUT = 512
NDICT = 100
H = W = 56
HP = WP = 58  # padded
ROWS_PER_TILE = 8
N_TILES = H // ROWS_PER_TILE  # 7
FREE = ROWS_PER_TILE * W  # 448
S = 3  # lookup sparsity

TRACE = False  # set by test.py to get a profile
_LAST_RESULTS = {}  # test.py reads exec_time_ns from here


def split12(a):
    """Exact split a = hi + lo with <=12 significant bits each (a ~ N(0,1),
    so no denormal/overflow concerns).  Both halves pass through the f32r
    12-bit RNE rounding unchanged."""
    a = np.ascontiguousarray(a, dtype=np.float32)
    hi = (a.view(np.uint32) & np.uint32(0xFFFFF000)).view(np.float32)
    lo = (a - hi).astype(np.float32)
    return hi, lo


def _build_program():
    import concourse.bacc as bacc
    import concourse.mybir as mybir
    import concourse.tile as tile

    f32 = mybir.dt.float32
    f32r = mybir.dt.float32r

    nc = bacc.Bacc("TRN2", target_bir_lowering=False, debug=False)

    xh_d = nc.dram_tensor("xh", (IMGS_PER_CORE, CIN, HP, WP), f32,
                          kind="ExternalInput")
    xl_d = nc.dram_tensor("xl", (IMGS_PER_CORE, CIN, HP, WP), f32,
                          kind="ExternalInput")
    wh_d = nc.dram_tensor("wh", (128, 2 * 9 * NDICT), f32, kind="ExternalInput")
    wl_d = nc.dram_tensor("wl", (128, 2 * 9 * NDICT), f32, kind="ExternalInput")
    mh_d = nc.dram_tensor("mh", (NDICT, COUT), f32, kind="ExternalInput")
    ml_d = nc.dram_tensor("ml", (NDICT, COUT), f32, kind="ExternalInput")
    out_d = nc.dram_tensor("out", (IMGS_PER_CORE, COUT, H, W), f32,
                           kind="ExternalOutput")

    # row chunks of the padded input: first 10 rows, then 6x8 -- tile t only
    # needs chunks 0..t so compute starts after the first chunk lands
    row_chunks = [(0, 10)] + [(10 + 8 * k, 8) for k in range(6)]

    with tile.TileContext(nc) as tc:
        with (
            tc.tile_pool(name="consts", bufs=1) as consts,
            tc.tile_pool(name="xpool", bufs=1) as xpool,
            tc.tile_pool(name="ypool", bufs=3) as ypool,
            tc.tile_pool(name="opool", bufs=8) as opool,
            tc.tile_pool(name="psum_y", bufs=2, space="PSUM") as psum_y_pool,
            tc.tile_pool(name="psum_o", bufs=4, space="PSUM") as psum_o_pool,
        ):
            wh_sb = consts.tile([128, 2 * 9 * NDICT], f32r)
            nc.sync.dma_start(wh_sb[:], wh_d[:].bitcast(f32r))
            wl_sb = consts.tile([128, 2 * 9 * NDICT], f32r)
            nc.sync.dma_start(wl_sb[:], wl_d[:].bitcast(f32r))
            mh_sb = consts.tile([NDICT, COUT], f32r)
            nc.sync.dma_start(mh_sb[:], mh_d[:].bitcast(f32r))
            ml_sb = consts.tile([NDICT, COUT], f32r)
            nc.sync.dma_start(ml_sb[:], ml_d[:].bitcast(f32r))

            # [128 cin-in-block, img, cblk, hp, wp]
            xh_sb = xpool.tile([128, IMGS_PER_CORE, 2, HP, WP], f32r,
                               tag="xh_sb")
            xl_sb = xpool.tile([128, IMGS_PER_CORE, 2, HP, WP], f32r,
                               tag="xl_sb")
            xh_v = xh_d.rearrange("i (b c) h w -> c i b h w", c=128)
            xl_v = xl_d.rearrange("i (b c) h w -> c i b h w", c=128)
            for img in range(IMGS_PER_CORE):
                for r0, nr in row_chunks:
                    for cb in range(2):
                        nc.sync.dma_start(
                            xh_sb[:, img, cb, r0:r0 + nr, :],
                            xh_v[:, img, cb, r0:r0 + nr, :].bitcast(f32r))
                        nc.sync.dma_start(
                            xl_sb[:, img, cb, r0:r0 + nr, :],
                            xl_v[:, img, cb, r0:r0 + nr, :].bitcast(f32r))

            out_v = out_d.rearrange("i (b o) h w -> i b o (h w)", o=128)

            n_mm = 3 * 18

            def emit_conv(img, h0):
                py = psum_y_pool.tile([NDICT, FREE], f32)
                k = 0
                for cb in range(2):
                    for ti in range(3):
                        for tj in range(3):
                            tap = slice((cb * 9 + ti * 3 + tj) * NDICT,
                                        (cb * 9 + ti * 3 + tj + 1) * NDICT)
                            rh = (slice(None), img, cb,
                                  slice(h0 + ti, h0 + ti + ROWS_PER_TILE),
                                  slice(tj, tj + W))
                            for lhsT, rhs in (
                                (wh_sb[:, tap], xh_sb[rh]),
                                (wh_sb[:, tap], xl_sb[rh]),
                                (wl_sb[:, tap], xh_sb[rh]),
                            ):
                                nc.tensor.matmul(
                                    py[:], lhsT, rhs,
                                    start=(k == 0), stop=(k == n_mm - 1))
                                k += 1
                return py

            def emit_mix(py, img, h0):
                # Veltkamp split y = yh + yl into 12-bit halves (pure fp32
                # arithmetic; values are exactly f32r-representable so the
                # matmul's internal rounding is the identity)
                t_sb = ypool.tile([NDICT, FREE], f32, tag="t")
                big = ypool.tile([NDICT, FREE], f32, tag="big")
                yh = ypool.tile([NDICT, FREE], f32r, tag="yh")
                yl = ypool.tile([NDICT, FREE], f32r, tag="yl")
                nc.scalar.mul(t_sb[:], py[:], 4097.0)
                nc.vector.tensor_sub(big[:], t_sb[:], py[:])
                nc.vector.tensor_sub(yh[:], t_sb[:], big[:])
                nc.vector.tensor_sub(yl[:], py[:], yh[:])
                for ob in range(4):
                    obs = slice(ob * 128, (ob + 1) * 128)
                    po = psum_o_pool.tile([128, FREE], f32)
                    nc.tensor.matmul(po[:], mh_sb[:, obs], yh[:],
                                     start=True, stop=False)
                    nc.tensor.matmul(po[:], ml_sb[:, obs], yh[:],
                                     start=False, stop=False)
                    nc.tensor.matmul(po[:], mh_sb[:, obs], yl[:],
                                     start=False, stop=True)
                    o_sb = opool.tile([128, FREE], f32)
                    if ob % 2 == 0:
                        nc.vector.tensor_copy(o_sb[:], po[:])
                    else:
                        nc.scalar.copy(o_sb[:], po[:])
                    nc.sync.dma_start(
                        out_v[img, ob, :, h0 * W:h0 * W + FREE], o_sb[:])

            # software-pipeline by one tile: PE runs tile i's conv while
            # ACT/DVE run tile i-1's Veltkamp split, so the mix matmuls are
            # ready when PE gets to them
            pending = None
            for img in range(IMGS_PER_CORE):
                for t in range(N_TILES):
                    h0 = t * ROWS_PER_TILE
                    py = emit_conv(img, h0)
                    if pending is not None:
                        emit_mix(*pending)
                    pending = (py, img, h0)
            emit_mix(*pending)

    nc.compile()
    return nc


_NC_CACHE = None


def kernel(x, dictionary, lookup_indices, lookup_coefficients):
    global _NC_CACHE
    from concourse import bass_utils

    x = np.asarray(x, dtype=np.float32)
    dictionary = np.asarray(dictionary, dtype=np.float32)
    idx = np.asarray(lookup_indices).astype(np.int64)
    coef = np.asarray(lookup_coefficients, dtype=np.float32)

    # M^T[d, o] = sum_s coeff[o, s] * [idx[o, s] == d]
    mt = np.zeros((NDICT, COUT), np.float32)
    np.add.at(mt, (idx.reshape(-1),
                   np.repeat(np.arange(COUT), S)), coef.reshape(-1))

    # wt[c_in_block, (cblk, ti, tj, d)] = dictionary[d, cblk*128+c, ti, tj]
    wt = np.ascontiguousarray(
        dictionary.reshape(NDICT, 2, 128, 3, 3).transpose(2, 1, 3, 4, 0)
    ).reshape(128, 2 * 9 * NDICT)
    wh, wl = split12(wt)
    mh, ml = split12(mt)

    xp = np.pad(x, ((0, 0), (0, 0), (1, 1), (1, 1)))
    xp = np.ascontiguousarray(
        xp.reshape(N_CORES, IMGS_PER_CORE, CIN, HP, WP))
    xh, xl = split12(xp)

    if _NC_CACHE is None:
        _NC_CACHE = _build_program()
    nc = _NC_CACHE

    in_maps = [{"xh": xh[i], "xl": xl[i], "wh": wh, "wl": wl,
                "mh": mh, "ml": ml} for i in range(N_CORES)]
    try:
        res = bass_utils.run_bass_kernel_spmd(
            nc, in_maps, core_ids=list(range(N_CORES)), trace=TRACE)
    except ModuleNotFoundError:
        # no axon NTFF profile hook in this environment
        res = bass_utils.run_bass_kernel_spmd(
            nc, in_maps, core_ids=list(range(N_CORES)), trace=False)
    _LAST_RESULTS["res"] = res

    out = np.concatenate([r["out"] for r in res.results], axis=0)
    return out.reshape(16, COUT, H, W)

